# revision 1
# baseline (speedup 1.0000x reference)
"""TreeLSTM-style DERNN kernel for Trainium2 (Bass/Tile), 8-core data-parallel.

Strategy
--------
- Shard the 512 trees across 8 cores (64 trees/core); replicate the small
  parameters and the 50000x300 embedding table.
- Each tree is a complete binary tree of 127 nodes. Process levels
  bottom-up (depth 6 leaves -> depth 0 root). Nodes are reordered
  host-side into level-major, tree-major order so that the two children
  of parent position p sit at child positions 2p, 2p+1: segment_sum
  becomes a stride-2 column add.
- On-chip layout is transposed: [feature (partitions), node (free dim)].
  Weights stay stationary on the PE; node activations stream as the
  moving operand.
- Embedding rows are gathered with indirect DMA ([<=128 nodes, 300] rows,
  cast fp32->bf16 in the DMA), transposed to [E, nodes] with the DMA
  xbar transpose (128x128 bf16 blocks, SBUF->SBUF) - the PE does only
  matmuls.
- dep-type terms (q @ D.T gathered by dep id) are K=10 one-hot matmuls
  that accumulate into the same PSUM as the main projections. All biases
  are folded host-side into those tables (bf into qDf, biu/2 into qDiu
  since every parent has exactly 2 children, leaf constant as an ACT
  per-partition bias).
- All matmul operands are bf16 (fast LDWEIGHTS / FWL); PSUM stays fp32.
"""

import os
import sys

import numpy as np

for _p in ("/opt/trn_rl_repo", "/root/.axon_site/_ro/trn_rl_repo"):
    if _p not in sys.path and os.path.isdir(_p):
        sys.path.append(_p)

B, N, H, E, V, Q = 512, 127, 256, 300, 50000, 10
NCORES = 8
CH = 512  # parent chunk size
EP = 384  # E padded to xbar multiple of 128


def _plan(BT):
    """Static per-core schedule: level sizes, node offsets, gather columns."""
    LS = [BT * (64 >> lv) for lv in range(7)]  # nodes at level lv (lv0=leaves)
    NOFF = [0]
    for lv in range(7):
        NOFF.append(NOFF[-1] + LS[lv])
    POFF = [0]  # parent-block offsets (for deppair), levels 1..6
    for lv in range(1, 7):
        POFF.append(POFF[-1] + LS[lv])
    chunks = []
    gcol = 0
    for lv in range(7):
        lvchunks = []
        off = 0
        while off < LS[lv]:
            pcount = min(CH, LS[lv] - off)
            gsubs = []
            r = 0
            while r < pcount:
                rows = min(128, pcount - r)
                gsubs.append((gcol, 1, rows))
                gcol += 1
                r += rows
            lvchunks.append((off, pcount, gsubs))
            off += pcount
        chunks.append(lvchunks)
    return LS, NOFF, POFF, chunks, gcol


def _perm(BT):
    """Map level-major position -> flat (tree*127 + node) index."""
    out = []
    for lv in range(7):
        d = 6 - lv
        base = (1 << d) - 1
        cnt = 1 << d
        node = base + np.arange(cnt)
        out.append((np.arange(BT)[:, None] * 127 + node[None, :]).reshape(-1))
    return np.concatenate(out)


def build_nc(BT):
    import concourse.bacc as bacc
    import concourse.bass as bass
    import concourse.mybir as mybir
    import concourse.tile as tile

    f32 = mybir.dt.float32
    bf16 = mybir.dt.bfloat16
    i32 = mybir.dt.int32
    AF = mybir.ActivationFunctionType

    LS, NOFF, POFF, chunks, G = _plan(BT)
    NN = BT * 127
    NPAR = BT * 63

    nc = bacc.Bacc("TRN2", target_bir_lowering=False, debug=False,
                   num_devices=NCORES)
    emb_d = nc.declare_dram_parameter("emb", [V, E], f32, isOutput=False)
    tok_d = nc.declare_dram_parameter("tok", [128, G], i32, isOutput=False)
    doh_d = nc.declare_dram_parameter("depoh", [10, NN], bf16, isOutput=False)
    dpr_d = nc.declare_dram_parameter("deppair", [10, NPAR], bf16,
                                      isOutput=False)
    wa0_d = nc.declare_dram_parameter("wa0", [128, 768], bf16, isOutput=False)
    wa1_d = nc.declare_dram_parameter("wa1", [128, 768], bf16, isOutput=False)
    wa2_d = nc.declare_dram_parameter("wa2", [44, 768], bf16, isOutput=False)
    qdiu_d = nc.declare_dram_parameter("qdiu", [10, 512], bf16, isOutput=False)
    wa2x_d = nc.declare_dram_parameter("wa2x", [74, 512], bf16,
                                       isOutput=False)
    lfb_d = nc.declare_dram_parameter("leafb", [128, 4], f32, isOutput=False)
    iot_d = nc.declare_dram_parameter("iota", [128, 32], mybir.dt.int16,
                                      isOutput=False)
    small_pc = sorted({pc for lvc in chunks for (_, pc, _) in lvc if pc < 128})
    iop_d = {pc: nc.declare_dram_parameter(f"iotap{pc}", [128, 8],
                                           mybir.dt.int16, isOutput=False)
             for pc in small_pc}
    u0_d = nc.declare_dram_parameter("u0", [128, 768], bf16, isOutput=False)
    u1_d = nc.declare_dram_parameter("u1", [128, 768], bf16, isOutput=False)
    qdf_d = nc.declare_dram_parameter("qdf", [10, 256], bf16, isOutput=False)
    out_d = nc.declare_dram_parameter("out", [BT, 256], f32, isOutput=True)

    def dup2(ap):
        s = list(ap.shape)
        return ap.unsqueeze(2).to_broadcast(s + [2])

    def mm(o, lhsT, rhs, start, stop):
        nc.tensor.matmul(o, lhsT, rhs, start=start, stop=stop)

    with tile.TileContext(nc) as tc:
        with (
            tc.tile_pool(name="const", bufs=1) as const,
            tc.tile_pool(name="xnat", bufs=10) as xnat,
            tc.tile_pool(name="xa", bufs=3) as xap,
            tc.tile_pool(name="trps", bufs=2, space="PSUM") as trps,
            tc.tile_pool(name="fps", bufs=3, space="PSUM") as fps,
            tc.tile_pool(name="iups", bufs=3, space="PSUM") as iups,
            tc.tile_pool(name="work", bufs=3) as work,
        ):
            def load(dram, shape, dtype):
                t = const.tile(shape, dtype, name=f"ld_{dram.name}")
                nc.sync.dma_start(out=t[:], in_=dram.ap())
                return t

            wa0_sb = load(wa0_d, [128, 768], bf16)
            wa1_sb = load(wa1_d, [128, 768], bf16)
            wa2_sb = load(wa2_d, [44, 768], bf16)
            wa2x_sb = load(wa2x_d, [74, 512], bf16)
            lfb_sb = load(lfb_d, [128, 4], f32)
            u0_sb = load(u0_d, [128, 768], bf16)
            u1_sb = load(u1_d, [128, 768], bf16)
            qdf_sb = load(qdf_d, [10, 256], bf16)
            tok_sb = load(tok_d, [128, G], i32)
            iot_sb = load(iot_d, [128, 32], mybir.dt.int16)
            iop_sb = {pc: load(d, [128, 8], mybir.dt.int16)
                      for pc, d in iop_d.items()}
            doh_sb = load(doh_d, [10, NN], bf16)

            ident = const.tile([128, 128], bf16)
            from concourse.masks import make_identity
            make_identity(nc, ident[:])

            hbig = [const.tile([128, LS[0]], bf16, name=f"hbig{m}")
                    for m in range(2)]
            hsml = [const.tile([128, LS[1]], bf16, name=f"hsml{m}")
                    for m in range(2)]
            HD = [hbig, hsml, hbig, hsml, hbig, hsml, hbig]

            for lv in range(7):
                hdst = HD[lv]
                hch = HD[lv - 1] if lv > 0 else None
                for (poff, pcount, gsubs) in chunks[lv]:
                    # --- gather embedding rows into stripe layout, bf16 ---
                    nst = (pcount + 127) // 128
                    NI = nst * 128
                    xc = xap.tile([128, 4 * 384], bf16, tag="xc")
                    xcv = xc[:].rearrange("p (k e) -> p k e", k=4)
                    if pcount < 128:
                        nc.vector.memset(xc[:, 0:384], 0)
                    else:
                        nc.vector.memset(xcv[:, 0:nst, E:384], 0)
                    for k, (col, nblk, rows) in enumerate(gsubs):
                        xf = xnat.tile([128, E], f32, tag="xnatf")
                        nc.gpsimd.indirect_dma_start(
                            out=xf[0:rows, 0:E],
                            out_offset=None,
                            in_=emb_d.ap(),
                            in_offset=bass.IndirectOffsetOnAxis(
                                ap=tok_sb[0:rows, col:col + 1], axis=0),
                        )
                        nc.vector.tensor_copy(
                            xc[0:rows, k * 384:k * 384 + E], xf[0:rows, 0:E])
                    # --- transpose to [E, nodes]: PE at lv0 (PE idle
                    # there), SWDGE transpose-gather at lv>=1 (PE busy) ---
                    xaT = xap.tile([128, 3 * 512], bf16, tag="xaT")
                    if lv == 0:
                        for eb in range(3):
                            w = 128 if eb < 2 else E - 256
                            trp = trps.tile([128, 512], bf16, tag="trps")
                            for k, (col, nblk, rows) in enumerate(gsubs):
                                nc.tensor.transpose(
                                    out=trp[0:w, k * 128:k * 128 + rows],
                                    in_=xcv[0:rows, k,
                                            eb * 128:eb * 128 + w],
                                    identity=ident[0:rows, 0:rows],
                                )
                            if eb == 0:
                                nc.scalar.copy(
                                    out=xaT[0:w, eb * NI:eb * NI + pcount],
                                    in_=trp[0:w, 0:pcount])
                            else:
                                nc.vector.tensor_copy(
                                    xaT[0:w, eb * NI:eb * NI + pcount],
                                    trp[0:w, 0:pcount])
                    else:
                        idxs = iop_sb[pcount][:, 0:8] if pcount < 128 else \
                            iot_sb[:, 0:NI // 16]
                        nc.gpsimd.dma_gather(
                            out_ap=xaT[:, 0:3 * NI].rearrange(
                                "p (b n) -> p b n", b=3),
                            in_ap=xc[:, 0:nst * 384],
                            idxs_ap=idxs,
                            num_idxs=NI,
                            num_idxs_reg=pcount,
                            elem_size=384,
                            transpose=True,
                            sbuf_tokens_per_rank=128,
                            sbuf_free_dim_per_rank=768,
                        )
                    xa0 = xaT[:, 0 * NI:0 * NI + pcount]
                    xa1 = xaT[:, 1 * NI:1 * NI + pcount]
                    xa2 = xaT[:, 2 * NI:2 * NI + pcount]
                    if lv > 0:
                        po = POFF[lv - 1] + poff
                        nc.sync.dma_start(
                            out=xaT[64:74, 2 * NI:2 * NI + pcount],
                            in_=dpr_d.ap()[:, po:po + pcount])

                    fsum = None
                    hs = None
                    if lv > 0:
                        # --- forget gates over the 2*pcount children ---
                        ccount = 2 * pcount
                        nhalf = (ccount + 511) // 512
                        choff = NOFF[lv - 1] + 2 * poff
                        hcol = 2 * poff
                        hc = [hch[m][:, hcol:hcol + ccount] for m in range(2)]
                        fe = [work.tile([128, 2 * CH], bf16, tag=f"fe{m}",
                                        name=f"fe{m}")
                              for m in range(2)]
                        for m in range(2):
                            mc = slice(m * 128, (m + 1) * 128)
                            for hf in range(nhalf):
                                cw = min(512, ccount - hf * 512)
                                cs = slice(hf * 512, hf * 512 + cw)
                                ps = slice(hf * 256, hf * 256 + cw // 2)
                                fp = fps.tile([128, 512], f32, tag="fps")
                                o = fp[:, 0:cw]
                                mm(o, wa0_sb[:, mc], dup2(xa0[:, ps]),
                                   start=True, stop=False)
                                mm(o, wa1_sb[:, mc], dup2(xa1[:, ps]),
                                   start=False, stop=False)
                                mm(o, wa2_sb[:, mc], dup2(xa2[0:44, ps]),
                                   start=False, stop=False)
                                mm(o, u0_sb[:, mc], hc[0][:, cs],
                                   start=False, stop=False)
                                mm(o, u1_sb[:, mc], hc[1][:, cs],
                                   start=False, stop=False)
                                mm(o, qdf_sb[:, mc],
                                   doh_sb[:, choff + hf * 512:
                                          choff + hf * 512 + cw],
                                   start=False, stop=True)
                                nc.scalar.activation(fe[m][:, cs], o,
                                                     AF.Sigmoid)
                        fsum = work.tile([128, 2 * CH], bf16, tag="fsum")
                        hs = []
                        for m in range(2):
                            fh = work.tile([128, 2 * CH], bf16, tag=f"fh{m}")
                            nc.vector.tensor_mul(fh[:, 0:ccount],
                                                 fe[m][:, 0:ccount], hc[m])
                            nc.vector.tensor_add(
                                fsum[:, m * CH:m * CH + pcount],
                                fh[:, 0:ccount:2], fh[:, 1:ccount:2])
                            hsm = work.tile([128, CH], bf16, tag=f"hs{m}")
                            nc.vector.tensor_add(hsm[:, 0:pcount],
                                                 hc[m][:, 0::2], hc[m][:, 1::2])
                            hs.append(hsm)

                    # --- iu projections, one PSUM bank at a time ---
                    si = work.tile([128, 2 * CH], bf16, tag="si")
                    tu = work.tile([128, 2 * CH], bf16, tag="tu")
                    for mi in range(4):
                        wc = slice(256 + mi * 128, 256 + (mi + 1) * 128)
                        wc2 = slice(mi * 128, (mi + 1) * 128)
                        ps = iups.tile([128, 512], f32, tag="iups")
                        o = ps[:, 0:pcount]
                        mm(o, wa0_sb[:, wc], xa0[:, :],
                           start=True, stop=False)
                        mm(o, wa1_sb[:, wc], xa1[:, :],
                           start=False, stop=False)
                        if lv == 0:
                            mm(o, wa2_sb[:, wc], xa2[0:44, :],
                               start=False, stop=True)
                        else:
                            mm(o, wa2x_sb[:, wc2], xa2[0:74, :],
                               start=False, stop=False)
                            mm(o, u0_sb[:, wc], hs[0][:, 0:pcount],
                               start=False, stop=False)
                            mm(o, u1_sb[:, wc], hs[1][:, 0:pcount],
                               start=False, stop=True)
                        dst = si if mi < 2 else tu
                        dsl = slice((mi % 2) * CH, (mi % 2) * CH + pcount)
                        fn = AF.Sigmoid if mi < 2 else AF.Tanh
                        bias = lfb_sb[:, mi:mi + 1] if lv == 0 else 0.0
                        nc.scalar.activation(dst[:, dsl], o, fn, bias=bias)
                    g = work.tile([128, 2 * CH], bf16, tag="g")
                    g2 = work.tile([128, 2 * CH], bf16, tag="g2")
                    for m in range(2):
                        sl = slice(m * CH, m * CH + pcount)
                        nc.vector.tensor_mul(g[:, sl], si[:, sl], tu[:, sl])
                        pre = g
                        if lv > 0:
                            nc.vector.tensor_add(g2[:, sl], g[:, sl],
                                                 fsum[:, sl])
                            pre = g2
                        nc.scalar.activation(hdst[m][:, poff:poff + pcount],
                                             pre[:, sl], AF.Tanh)

            # --- transpose root h back to [tree, H] and store ---
            roots = LS[6]
            trp = trps.tile([128, 512], bf16, tag="trps")
            for m in range(2):
                nc.tensor.transpose(
                    out=trp[0:roots, m * 128:(m + 1) * 128],
                    in_=HD[6][m][:, 0:roots],
                    identity=ident[:, :],
                )
            outsb = const.tile([BT, 256], f32)
            nc.scalar.copy(out=outsb[:, :], in_=trp[0:roots, 0:256])
            nc.sync.dma_start(out=out_d.ap(), in_=outsb[:])

    nc.compile()
    return nc


def prep_inputs(tokens, dep, idx2vec, q, W, U, D, b, BT):
    """Host-side prep: returns per-core input maps."""
    import ml_dtypes

    bf = ml_dtypes.bfloat16
    tokens = np.asarray(tokens, np.int32)
    dep = np.asarray(dep, np.int32)
    idx2vec = np.ascontiguousarray(np.asarray(idx2vec, np.float32))
    q = np.asarray(q, np.float32)
    W = np.asarray(W, np.float32)
    U = np.asarray(U, np.float32)
    D = np.asarray(D, np.float32)
    b = np.asarray(b, np.float32)

    LS, NOFF, POFF, chunks, G = _plan(BT)
    NN = BT * 127
    NPAR = BT * 63
    perm = _perm(BT)

    WT = np.ascontiguousarray(W.T)  # [300, 768]
    UT = np.ascontiguousarray(U.T)  # [256, 768]
    qD = q @ D.T  # [10, 768]
    qdf = np.ascontiguousarray(qD[:, :256] + b[None, :256])
    qdiu = qD[:, 256:] + b[None, 256:] / 2.0  # [10, 512]
    leafconst = q[-1] @ D[256:].T + b[256:]  # [512]

    wa0 = np.ascontiguousarray(WT[0:128]).astype(bf)
    wa1 = np.ascontiguousarray(WT[128:256]).astype(bf)
    wa2 = np.ascontiguousarray(WT[256:300]).astype(bf)
    leafb = np.ascontiguousarray(leafconst.reshape(4, 128).T)

    wa2x = np.zeros((74, 512), np.float32)
    wa2x[0:44] = WT[256:300, 256:768]
    wa2x[64:74] = qdiu
    iota = np.zeros((16, 32), np.int16)
    for i in range(512):
        iota[i % 16, i // 16] = i
    iota = np.tile(iota, (8, 1))
    shared = dict(emb=idx2vec, iota=iota,
                  wa0=wa0, wa1=wa1, wa2=wa2,
                  qdiu=np.ascontiguousarray(qdiu).astype(bf),
                  wa2x=wa2x.astype(bf), leafb=leafb,
                  u0=np.ascontiguousarray(UT[0:128]).astype(bf),
                  u1=np.ascontiguousarray(UT[128:256]).astype(bf),
                  qdf=qdf.astype(bf))

    for pc in sorted({p for lvc in chunks for (_, p, _) in lvc if p < 128}):
        iop = np.full((16, 8), -1, np.int16)
        for i in range(pc):
            iop[i % 16, i // 16] = i
        shared[f"iotap{pc}"] = np.tile(iop, (8, 1))
    ncores = tokens.shape[0] // BT
    per_core = []
    for c in range(ncores):
        tsh = tokens[c * BT:(c + 1) * BT].reshape(-1)[perm]  # [NN] level-major
        dsh = dep[c * BT:(c + 1) * BT].reshape(-1)[perm]
        tok2d = np.zeros((128, G), np.int32)
        for lv in range(7):
            for (poff, pcount, gsubs) in chunks[lv]:
                base = NOFF[lv] + poff
                r = 0
                for (col, nblk, rows) in gsubs:
                    for k in range(nblk):
                        tok2d[0:rows, col + k] = tsh[base + r:base + r + rows]
                        r += rows
        depoh = (dsh[None, :] == np.arange(10)[:, None]).astype(np.float32)
        deppair = np.zeros((10, NPAR), np.float32)
        for lv in range(1, 7):
            chld = depoh[:, NOFF[lv - 1]:NOFF[lv - 1] + LS[lv - 1]]
            deppair[:, POFF[lv - 1]:POFF[lv - 1] + LS[lv]] = (
                chld.reshape(10, LS[lv], 2).sum(-1))
        m = dict(shared)
        m.update(tok=tok2d, depoh=np.ascontiguousarray(depoh).astype(bf),
                 deppair=deppair.astype(bf))
        per_core.append(m)
    return per_core


_NC_CACHE = {}
TRACE = False
LAST = None


def _get_nc(BT):
    if BT not in _NC_CACHE:
        _NC_CACHE[BT] = build_nc(BT)
    return _NC_CACHE[BT]


def kernel(tokens, dep, idx2vec, q, W, U, D, b):
    global LAST
    from concourse.bass_utils import run_bass_kernel_spmd

    BT = B // NCORES
    nc = _get_nc(BT)
    in_maps = prep_inputs(tokens, dep, idx2vec, q, W, U, D, b, BT)
    res = run_bass_kernel_spmd(nc, in_maps, list(range(NCORES)), trace=TRACE)
    LAST = res
    return np.concatenate([res.results[i]["out"] for i in range(NCORES)],
                          axis=0)



# revision 9
# speedup vs baseline: 1.3345x; 1.3345x over previous
"""TreeLSTM-style DERNN kernel for Trainium2 (Bass/Tile), 8-core data-parallel.

Strategy (v2)
-------------
- Shard the 512 trees across 8 cores (64 trees/core); each tree is a
  complete binary tree of 127 nodes, processed level-synchronously
  (leaves -> root).
- Host-side prep does the embedding gather + transpose: x arrives as
  fp8 [feature, node] streams in level-major order, with each level laid
  out [left-children | right-children] so pair reductions are contiguous
  adds and the f-gate can reuse the parent's x stream for both halves.
- All x-side projections run as fp8 e4m3 DoubleRow matmuls (2 K-tiles
  per pass, 0.5 cyc/row). The dep-type terms are folded into the K
  remainder tile (rows 44:54 = one-hot / pair-sum one-hot), the leaf
  iu constant into row 54, and all biases into the host-prepped weight
  tiles, so there are no separate dep matmuls and no ACT biases.
- U·h terms stay bf16 (fp8 h fails accuracy) and accumulate into the
  same PSUM region as the x projections; gates activate directly from
  PSUM. PSUM: 2 pools x 2 bufs x 2 banks = all 8 banks, giving depth-2
  chunk pipelining so the PE never drains (pstate stays at 2.4 GHz).
"""

import os
import sys

import numpy as np

for _p in ("/opt/trn_rl_repo", "/root/.axon_site/_ro/trn_rl_repo"):
    if _p not in sys.path and os.path.isdir(_p):
        sys.path.append(_p)

B, N, H, E, V, Q = 512, 127, 256, 300, 50000, 10
NCORES = 8
BT = B // NCORES          # trees per core
NN = BT * 127             # nodes per core
CN = BT * 126             # child nodes per core (levels 0..5)
LS = [BT * (64 >> lv) for lv in range(7)]    # level sizes, lv0 = leaves
NOFF = [0]
for _lv in range(7):
    NOFF.append(NOFF[-1] + LS[_lv])

PCH = 256    # parent chunk (internal levels)
LCH = 512    # leaf chunk

USE_BCAST = True   # single f-x matmul with broadcast-halves AP


def _order():
    """Level-major node order; within each level [left kids | right kids]
    of the previous (parent) level's order. Returns flat node ids."""
    t = np.arange(BT) * 127
    ords = {6: t.copy()}                     # roots
    for lv in range(5, -1, -1):
        par = ords[lv + 1]
        tt = par // 127
        n = par % 127
        left = tt * 127 + 2 * n + 1
        right = tt * 127 + 2 * n + 2
        ords[lv] = np.concatenate([left, right])
    return np.concatenate([ords[lv] for lv in range(7)])


PERM = _order()


def build_nc():
    import concourse.bacc as bacc
    import concourse.bass as bass  # noqa: F401
    import concourse.mybir as mybir
    import concourse.tile as tile

    f32 = mybir.dt.float32
    bf16 = mybir.dt.bfloat16
    f8 = mybir.dt.float8e4
    AF = mybir.ActivationFunctionType
    DR = mybir.MatmulPerfMode.DoubleRow

    nc = bacc.Bacc("TRN2", target_bir_lowering=False, debug=False,
                   num_devices=NCORES)

    xp_d = [nc.declare_dram_parameter(f"xp{lv}", [128, 2 * LS[lv]], f8,
                                      isOutput=False) for lv in range(7)]
    x2iu_d = [nc.declare_dram_parameter(f"x2iu{lv}", [33, 2 * LS[lv]], f8,
                                        isOutput=False) for lv in range(7)]
    x2f_d = [None] + [nc.declare_dram_parameter(
        f"x2f{lv}", [32, 2 * LS[lv - 1]], f8, isOutput=False)
        for lv in range(1, 7)]
    w_d = nc.declare_dram_parameter("wk", [128, 2 * 768], f8, isOutput=False)
    w2iu_d = nc.declare_dram_parameter("w2iu", [33, 2 * 512], f8,
                                       isOutput=False)
    w2f_d = nc.declare_dram_parameter("w2f", [32, 2 * 256], f8,
                                      isOutput=False)
    u_d = nc.declare_dram_parameter("uk", [128, 2 * 768], bf16,
                                    isOutput=False)
    out_d = nc.declare_dram_parameter("out", [128, 2 * BT], bf16,
                                      isOutput=True)

    with tile.TileContext(nc) as tc:
        with (
            tc.tile_pool(name="const", bufs=1) as const,
            tc.tile_pool(name="pa", bufs=2, space="PSUM") as pa,
            tc.tile_pool(name="pb", bufs=2, space="PSUM") as pb,
            tc.tile_pool(name="work", bufs=3) as work,
        ):
            def load(dram, shape, dtype):
                t = const.tile(shape, dtype, name=f"ld_{dram.name}")
                nc.sync.dma_start(out=t[:], in_=dram.ap())
                return t

            w_sb = load(w_d, [128, 2 * 768], f8)
            w2iu_sb = load(w2iu_d, [33, 2 * 512], f8)
            w2f_sb = load(w2f_d, [32, 2 * 256], f8)
            u_sb = load(u_d, [128, 2 * 768], bf16)
            xp_sb = [load(xp_d[lv], [128, 2 * LS[lv]], f8) for lv in range(7)]
            x2iu_sb = [load(x2iu_d[lv], [33, 2 * LS[lv]], f8)
                       for lv in range(7)]
            x2f_sb = [None] + [load(x2f_d[lv], [32, 2 * LS[lv - 1]], f8)
                               for lv in range(1, 7)]

            h_sb = [const.tile([128, 2 * LS[lv]], bf16, name=f"h{lv}")
                    for lv in range(7)]
            hs_sb = [None] + [const.tile([128, 2 * LS[lv]], bf16,
                                         name=f"hs{lv}")
                              for lv in range(1, 7)]
            fs_sb = [None] + [const.tile([128, 2 * LS[lv]], bf16,
                                         name=f"fs{lv}")
                              for lv in range(1, 7)]

            # k-tile views
            wv = w_sb[:].rearrange("p (k m) -> p k m", k=2)       # [128,2,768]
            w2iuv = w2iu_sb[:].rearrange("p (k m) -> p k m", k=2)  # [28,2,512]
            w2fv = w2f_sb[:].rearrange("p (k m) -> p k m", k=2)   # [28,2,256]
            uv = u_sb[:].rearrange("p (k m) -> p k m", k=2)       # [128,2,768]
            xpv = [xp_sb[lv][:].rearrange("p (k n) -> p k n", k=2)
                   for lv in range(7)]
            x2iuv = [x2iu_sb[lv][:].rearrange("p (k n) -> p k n", k=2)
                     for lv in range(7)]
            x2fv = [None] + [x2f_sb[lv][:].rearrange("p (k n) -> p k n", k=2)
                             for lv in range(1, 7)]

            def mm(o, lhsT, rhs, start, stop, dr=False):
                nc.tensor.matmul(o, lhsT, rhs, start=start, stop=stop,
                                 perf_mode=DR if dr else None)

            # ---------------- leaves (lv 0) ----------------
            L0 = LS[0]
            for p0 in range(0, L0, LCH):
                cw = min(LCH, L0 - p0)
                psI = pa.tile([128, 1024], f32, tag="psA")
                psU = pb.tile([128, 1024], f32, tag="psB")
                for m in range(4):
                    ps = psI if m < 2 else psU
                    o = ps[:, (m % 2) * 512:(m % 2) * 512 + cw]
                    mc = slice(256 + m * 128, 256 + (m + 1) * 128)
                    mm(o, wv[:, :, mc], xpv[0][:, :, p0:p0 + cw],
                       start=True, stop=False, dr=True)
                    mm(o, w2iuv[:, :, m * 128:(m + 1) * 128],
                       x2iuv[0][:, :, p0:p0 + cw],
                       start=False, stop=True, dr=True)
                si = work.tile([128, 1024], bf16, tag="siL")
                tu = work.tile([128, 1024], bf16, tag="tuL")
                for bk in range(2):
                    s = slice(bk * 512, bk * 512 + cw)
                    nc.scalar.activation(si[:, s], psI[:, s], AF.Sigmoid)
                    nc.scalar.activation(tu[:, s], psU[:, s], AF.Tanh)
                g = work.tile([128, 1024], bf16, tag="gL")
                nc.vector.tensor_mul(g[:, :], si[:, :], tu[:, :])
                gvw = g[:].rearrange("p (m c) -> p m c", m=2)[:, :, 0:cw]
                hovw = h_sb[0][:].rearrange("p (m c) -> p m c", m=2)[
                    :, :, p0:p0 + cw]
                nc.scalar.activation(hovw, gvw, AF.Tanh)

            # ---------------- internal levels ----------------
            for lv in range(1, 7):
                Lp, Lc = LS[lv], LS[lv - 1]
                hp = h_sb[lv - 1][:].rearrange("p (m c) -> p m c", m=2)
                hsv = hs_sb[lv][:].rearrange("p (m c) -> p m c", m=2)
                fsv = fs_sb[lv][:].rearrange("p (m c) -> p m c", m=2)
                hcv = h_sb[lv][:].rearrange("p (m c) -> p m c", m=2)

                # h_sum = h_left + h_right (contiguous halves)
                nc.vector.tensor_add(hsv[:, :, :],
                                     hp[:, :, 0:Lp], hp[:, :, Lp:Lc])

                # --- f gates, parent chunks (left+right kids in one psum) ---
                for p0 in range(0, Lp, PCH):
                    pw = min(PCH, Lp - p0)
                    psF = pb.tile([128, 1024], f32, tag="psB")
                    xsl = xpv[lv][:, :, NOFF[lv] - NOFF[lv] + p0:p0 + pw]
                    xsl = xpv[lv][:, :, p0:p0 + pw]
                    x2f_k = x2fv[lv][:, :, :].rearrange(
                        "p k (h c) -> p k h c", h=2)[:, :, :, p0:p0 + pw]
                    for m in range(2):
                        ov = psF[:, m * 512:(m + 1) * 512].rearrange(
                            "p (h c) -> p h c", h=2)[:, :, 0:pw]
                        mc = slice(m * 128, (m + 1) * 128)
                        if USE_BCAST:
                            xb = xsl.unsqueeze(2).to_broadcast(
                                [128, 2, 2, pw])
                            mm(ov, wv[:, :, mc], xb,
                               start=True, stop=False, dr=True)
                            mm(ov, w2fv[:, :, mc], x2f_k,
                               start=False, stop=False, dr=True)
                            for k in range(2):
                                hr = hp[:, k, 0:Lc].rearrange(
                                    "p (h c) -> p h c", h=2)[:, :, p0:p0 + pw]
                                mm(ov, uv[:, k, mc], hr,
                                   start=False, stop=(k == 1))
                        else:
                            for hh in range(2):
                                o = psF[:, m * 512 + hh * 256:
                                        m * 512 + hh * 256 + pw]
                                mm(o, wv[:, :, mc], xsl,
                                   start=True, stop=False, dr=True)
                                mm(o, w2fv[:, :, mc],
                                   x2f_k[:, :, hh, :],
                                   start=False, stop=False, dr=True)
                                for k in range(2):
                                    mm(o, uv[:, k, mc],
                                       hp[:, k, hh * Lp + p0:
                                          hh * Lp + p0 + pw],
                                       start=False, stop=(k == 1))
                    fe = work.tile([128, 1024], bf16, tag="fe")
                    for m in range(2):
                        iv = psF[:, m * 512:(m + 1) * 512].rearrange(
                            "p (h c) -> p h c", h=2)[:, :, 0:pw]
                        ev = fe[:, m * 512:(m + 1) * 512].rearrange(
                            "p (h c) -> p h c", h=2)[:, :, 0:pw]
                        nc.scalar.activation(ev, iv, AF.Sigmoid)
                    for m in range(2):
                        fhL = work.tile([128, 256], bf16, tag="fhL")
                        fhR = work.tile([128, 256], bf16, tag="fhR")
                        nc.vector.tensor_mul(
                            fhL[:, 0:pw], fe[:, m * 512:m * 512 + pw],
                            hp[:, m, p0:p0 + pw])
                        nc.vector.tensor_mul(
                            fhR[:, 0:pw],
                            fe[:, m * 512 + 256:m * 512 + 256 + pw],
                            hp[:, m, Lp + p0:Lp + p0 + pw])
                        nc.vector.tensor_add(
                            fsv[:, m, p0:p0 + pw], fhL[:, 0:pw],
                            fhR[:, 0:pw])

                # --- iu, parent chunks ---
                for p0 in range(0, Lp, PCH):
                    pw = min(PCH, Lp - p0)
                    psIU = pa.tile([128, 1024], f32, tag="psA")
                    for m in range(4):
                        o = psIU[:, m * 256:m * 256 + pw]
                        mc = slice(256 + m * 128, 256 + (m + 1) * 128)
                        mm(o, wv[:, :, mc], xpv[lv][:, :, p0:p0 + pw],
                           start=True, stop=False, dr=True)
                        mm(o, w2iuv[:, :, m * 128:(m + 1) * 128],
                           x2iuv[lv][:, :, p0:p0 + pw],
                           start=False, stop=False, dr=True)
                        for k in range(2):
                            mm(o, uv[:, k, mc], hsv[:, k, p0:p0 + pw],
                               start=False, stop=(k == 1))
                    si = work.tile([128, 512], bf16, tag="si")
                    tu = work.tile([128, 512], bf16, tag="tu")
                    sivw = si[:].rearrange("p (m c) -> p m c", m=2)[
                        :, :, 0:pw]
                    tuvw = tu[:].rearrange("p (m c) -> p m c", m=2)[
                        :, :, 0:pw]
                    piv = psIU[:, 0:512].rearrange(
                        "p (m c) -> p m c", m=2)[:, :, 0:pw]
                    puv = psIU[:, 512:1024].rearrange(
                        "p (m c) -> p m c", m=2)[:, :, 0:pw]
                    nc.scalar.activation(sivw, piv, AF.Sigmoid)
                    nc.scalar.activation(tuvw, puv, AF.Tanh)
                    g = work.tile([128, 512], bf16, tag="g")
                    g2 = work.tile([128, 512], bf16, tag="g2")
                    nc.vector.tensor_mul(g[:, :], si[:, :], tu[:, :])
                    gv = g[:].rearrange("p (m c) -> p m c", m=2)[:, :, 0:pw]
                    g2v = g2[:].rearrange("p (m c) -> p m c", m=2)[:, :, 0:pw]
                    nc.vector.tensor_add(g2v, gv, fsv[:, :, p0:p0 + pw])
                    nc.scalar.activation(hcv[:, :, p0:p0 + pw], g2v, AF.Tanh)

            # ---------------- roots -> output (host transposes) ----------
            nc.sync.dma_start(out=out_d.ap(), in_=h_sb[6][:, :])

    nc.compile()
    return nc


def prep_inputs(tokens, dep, idx2vec, q, W, U, D, b):
    """Host-side prep: per-core input maps with pre-gathered fp8 x streams."""
    import ml_dtypes

    bf = ml_dtypes.bfloat16
    f8 = ml_dtypes.float8_e4m3fn
    tokens = np.asarray(tokens, np.int32)
    dep = np.asarray(dep, np.int32)
    idx2vec = np.asarray(idx2vec, np.float32)
    q = np.asarray(q, np.float32)
    W = np.asarray(W, np.float32)
    U = np.asarray(U, np.float32)
    D = np.asarray(D, np.float32)
    b = np.asarray(b, np.float32)

    emb8 = idx2vec.astype(f8)

    WT = np.ascontiguousarray(W.T)            # [300, 768]
    UT = np.ascontiguousarray(U.T)            # [256, 768]
    qD = q @ D.T                              # [10, 768]
    qdiu = qD[:, 256:] + b[None, 256:] / 2.0  # [10, 512]
    qdf = qD[:, :256] + b[None, :256]         # [10, 256]
    leafconst = qD[-1, 256:] + b[256:]        # [512]

    wk = np.stack([WT[0:128], WT[128:256]])           # [2, 128, 768]
    wk = np.ascontiguousarray(wk.transpose(1, 0, 2)).astype(f8)

    def res8(v):
        a = v.astype(f8).astype(np.float32)
        return a, (v - a)

    w2iu = np.zeros((66, 512), np.float32)
    w2iu[0:44] = WT[256:300, 256:768]
    w2iu[44:54], w2iu[54:64] = res8(qdiu)
    w2iu[64], w2iu[65] = res8(leafconst[None, :])
    w2iu = np.ascontiguousarray(
        w2iu.reshape(2, 33, 512).transpose(1, 0, 2)).astype(f8)

    w2f = np.zeros((64, 256), np.float32)
    w2f[0:44] = WT[256:300, 0:256]
    w2f[44:54], w2f[54:64] = res8(qdf)
    w2f = np.ascontiguousarray(
        w2f.reshape(2, 32, 256).transpose(1, 0, 2)).astype(f8)

    uk = np.stack([UT[0:128], UT[128:256]])
    uk = np.ascontiguousarray(uk.transpose(1, 0, 2)).astype(bf)

    shared = dict(wk=wk.reshape(128, -1), w2iu=w2iu.reshape(33, -1),
                  w2f=w2f.reshape(32, -1), uk=uk.reshape(128, -1))

    P = PERM
    pnode = np.maximum((P % 127 - 1) // 2, 0) + (P // 127) * 127  # parent ids
    tt = P // 127
    n = P % 127
    lkid = tt * 127 + 2 * n + 1
    rkid = tt * 127 + 2 * n + 2
    internal = (n < 63)
    isleaf = ~internal

    per_core = []
    for c in range(NCORES):
        tokf = tokens[c * BT:(c + 1) * BT].reshape(-1)
        depf = dep[c * BT:(c + 1) * BT].reshape(-1)
        G8 = emb8[tokf[P]]                    # [NN, 300] fp8
        GP8 = emb8[tokf[pnode]]               # parent rows (for f gates)

        m = dict(shared)
        for lv in range(7):
            s = slice(NOFF[lv], NOFF[lv + 1])
            gs = G8[s]
            xp = np.stack([gs[:, 0:128].T, gs[:, 128:256].T])  # [2,128,L]
            m[f"xp{lv}"] = np.ascontiguousarray(
                xp.transpose(1, 0, 2)).reshape(128, -1)

            a = np.zeros((66, LS[lv]), np.float32)
            a[0:44] = gs[:, 256:300].T.astype(np.float32)
            if lv > 0:
                dl = depf[lkid[s]]
                dr = depf[rkid[s]]
                oh = (dl[None, :] == np.arange(10)[:, None]).astype(
                    np.float32)
                oh += (dr[None, :] == np.arange(10)[:, None])
                a[44:54] = oh
                a[54:64] = oh
            else:
                a[64] = 1.0
                a[65] = 1.0
            m[f"x2iu{lv}"] = np.ascontiguousarray(
                a.astype(f8).reshape(2, 33, -1).transpose(1, 0, 2)
            ).reshape(33, -1)

            if lv > 0:
                sc = slice(NOFF[lv - 1], NOFF[lv])
                gp = GP8[sc]
                af = np.zeros((64, LS[lv - 1]), np.float32)
                af[0:44] = gp[:, 256:300].T.astype(np.float32)
                dc = depf[P[sc]]
                af[44:54] = (dc[None, :] == np.arange(10)[:, None])
                af[54:64] = af[44:54]
                m[f"x2f{lv}"] = np.ascontiguousarray(
                    af.astype(f8).reshape(2, 32, -1).transpose(1, 0, 2)
                ).reshape(32, -1)
        per_core.append(m)
    return per_core


_NC_CACHE = {}
TRACE = False
LAST = None


def _get_nc():
    if "nc" not in _NC_CACHE:
        _NC_CACHE["nc"] = build_nc()
    return _NC_CACHE["nc"]


def kernel(tokens, dep, idx2vec, q, W, U, D, b):
    global LAST
    from concourse.bass_utils import run_bass_kernel_spmd

    nc = _get_nc()
    in_maps = prep_inputs(tokens, dep, idx2vec, q, W, U, D, b)
    res = run_bass_kernel_spmd(nc, in_maps, list(range(NCORES)), trace=TRACE)
    LAST = res
    outs = []
    for i in range(NCORES):
        arr = np.asarray(res.results[i]["out"], np.float32)  # [128, 2*BT]
        h = np.empty((BT, 256), np.float32)
        h[:, 0:128] = arr[:, 0:BT].T
        h[:, 128:256] = arr[:, BT:2 * BT].T
        outs.append(h)
    return np.concatenate(outs, axis=0)


# revision 10
# speedup vs baseline: 1.7281x; 1.2950x over previous
"""TreeLSTM-style DERNN kernel for Trainium2 (Bass/Tile), 8-core data-parallel.

Strategy (v2)
-------------
- Shard the 512 trees across 8 cores (64 trees/core); each tree is a
  complete binary tree of 127 nodes, processed level-synchronously
  (leaves -> root).
- Host-side prep does the embedding gather + transpose: x arrives as
  fp8 [feature, node] streams in level-major order, with each level laid
  out [left-children | right-children] so pair reductions are contiguous
  adds and the f-gate can reuse the parent's x stream for both halves.
- All x-side projections run as fp8 e4m3 DoubleRow matmuls (2 K-tiles
  per pass, 0.5 cyc/row). The dep-type terms are folded into the K
  remainder tile (rows 44:54 = one-hot / pair-sum one-hot), the leaf
  iu constant into row 54, and all biases into the host-prepped weight
  tiles, so there are no separate dep matmuls and no ACT biases.
- U·h terms stay bf16 (fp8 h fails accuracy) and accumulate into the
  same PSUM region as the x projections; gates activate directly from
  PSUM. PSUM: 2 pools x 2 bufs x 2 banks = all 8 banks, giving depth-2
  chunk pipelining so the PE never drains (pstate stays at 2.4 GHz).
"""

import os
import sys

import numpy as np

for _p in ("/opt/trn_rl_repo", "/root/.axon_site/_ro/trn_rl_repo"):
    if _p not in sys.path and os.path.isdir(_p):
        sys.path.append(_p)

B, N, H, E, V, Q = 512, 127, 256, 300, 50000, 10
NCORES = 8
BT = B // NCORES          # trees per core
NN = BT * 127             # nodes per core
CN = BT * 126             # child nodes per core (levels 0..5)
LS = [BT * (64 >> lv) for lv in range(7)]    # level sizes, lv0 = leaves
NOFF = [0]
for _lv in range(7):
    NOFF.append(NOFF[-1] + LS[_lv])

PCH = 256    # parent chunk (internal levels)
LCH = 512    # leaf chunk

USE_BCAST = True   # single f-x matmul with broadcast-halves AP


def _order():
    """Level-major node order; within each level [left kids | right kids]
    of the previous (parent) level's order. Returns flat node ids."""
    t = np.arange(BT) * 127
    ords = {6: t.copy()}                     # roots
    for lv in range(5, -1, -1):
        par = ords[lv + 1]
        tt = par // 127
        n = par % 127
        left = tt * 127 + 2 * n + 1
        right = tt * 127 + 2 * n + 2
        ords[lv] = np.concatenate([left, right])
    return np.concatenate([ords[lv] for lv in range(7)])


PERM = _order()


def build_nc():
    import concourse.bacc as bacc
    import concourse.bass as bass  # noqa: F401
    import concourse.mybir as mybir
    import concourse.tile as tile

    f32 = mybir.dt.float32
    bf16 = mybir.dt.bfloat16
    f8 = mybir.dt.float8e4
    AF = mybir.ActivationFunctionType
    DR = mybir.MatmulPerfMode.DoubleRow

    nc = bacc.Bacc("TRN2", target_bir_lowering=False, debug=False,
                   num_devices=NCORES)

    xp_d = [nc.declare_dram_parameter(f"xp{lv}", [128, 2 * LS[lv]], f8,
                                      isOutput=False) for lv in range(7)]
    x2iu_d = [nc.declare_dram_parameter(f"x2iu{lv}", [66, LS[lv]], f8,
                                        isOutput=False) for lv in range(7)]
    x2f_d = [None] + [nc.declare_dram_parameter(
        f"x2f{lv}", [64, LS[lv - 1]], f8, isOutput=False)
        for lv in range(1, 7)]
    w_d = nc.declare_dram_parameter("wk", [128, 2 * 768], f8, isOutput=False)
    w2iu_d = nc.declare_dram_parameter("w2iu", [66, 512], f8,
                                       isOutput=False)
    w2f_d = nc.declare_dram_parameter("w2f", [64, 256], f8,
                                      isOutput=False)
    u_d = nc.declare_dram_parameter("uk", [128, 2 * 768], bf16,
                                    isOutput=False)
    out_d = nc.declare_dram_parameter("out", [128, 2 * BT], bf16,
                                      isOutput=True)

    with tile.TileContext(nc) as tc:
        with (
            tc.tile_pool(name="const", bufs=1) as const,
            tc.tile_pool(name="pa", bufs=2, space="PSUM") as pa,
            tc.tile_pool(name="pb", bufs=2, space="PSUM") as pb,
            tc.tile_pool(name="work", bufs=3) as work,
        ):
            def load(dram, shape, dtype):
                t = const.tile(shape, dtype, name=f"ld_{dram.name}")
                nc.sync.dma_start(out=t[:], in_=dram.ap())
                return t

            w_sb = load(w_d, [128, 2 * 768], f8)
            w2iu_sb = load(w2iu_d, [66, 512], f8)
            w2f_sb = load(w2f_d, [64, 256], f8)
            u_sb = load(u_d, [128, 2 * 768], bf16)
            xp_sb = [load(xp_d[lv], [128, 2 * LS[lv]], f8) for lv in range(7)]
            x2iu_sb = [load(x2iu_d[lv], [66, LS[lv]], f8)
                       for lv in range(7)]
            x2f_sb = [None] + [load(x2f_d[lv], [64, LS[lv - 1]], f8)
                               for lv in range(1, 7)]

            h_sb = [const.tile([128, 2 * LS[lv]], bf16, name=f"h{lv}")
                    for lv in range(7)]
            hs_sb = [None] + [const.tile([128, 2 * LS[lv]], bf16,
                                         name=f"hs{lv}")
                              for lv in range(1, 7)]
            fs_sb = [None] + [const.tile([128, 2 * LS[lv]], bf16,
                                         name=f"fs{lv}")
                              for lv in range(1, 7)]

            # k-tile views
            wv = w_sb[:].rearrange("p (k m) -> p k m", k=2)       # [128,2,768]
            uv = u_sb[:].rearrange("p (k m) -> p k m", k=2)       # [128,2,768]
            xpv = [xp_sb[lv][:].rearrange("p (k n) -> p k n", k=2)
                   for lv in range(7)]

            def mm(o, lhsT, rhs, start, stop, dr=False):
                nc.tensor.matmul(o, lhsT, rhs, start=start, stop=stop,
                                 perf_mode=DR if dr else None)

            # ---------------- leaves (lv 0) ----------------
            L0 = LS[0]
            for p0 in range(0, L0, LCH):
                cw = min(LCH, L0 - p0)
                psI = pa.tile([128, 1024], f32, tag="psA")
                psU = pb.tile([128, 1024], f32, tag="psB")
                for m in range(4):
                    ps = psI if m < 2 else psU
                    o = ps[:, (m % 2) * 512:(m % 2) * 512 + cw]
                    mc = slice(256 + m * 128, 256 + (m + 1) * 128)
                    mm(o, wv[:, :, mc], xpv[0][:, :, p0:p0 + cw],
                       start=True, stop=False, dr=True)
                    mm(o, w2iu_sb[:, m * 128:(m + 1) * 128],
                       x2iu_sb[0][:, p0:p0 + cw],
                       start=False, stop=True)
                si = work.tile([128, 1024], bf16, tag="siL")
                tu = work.tile([128, 1024], bf16, tag="tuL")
                for bk in range(2):
                    s = slice(bk * 512, bk * 512 + cw)
                    nc.scalar.activation(si[:, s], psI[:, s], AF.Sigmoid)
                    nc.scalar.activation(tu[:, s], psU[:, s], AF.Tanh)
                g = work.tile([128, 1024], bf16, tag="gL")
                nc.vector.tensor_mul(g[:, :], si[:, :], tu[:, :])
                gvw = g[:].rearrange("p (m c) -> p m c", m=2)[:, :, 0:cw]
                hovw = h_sb[0][:].rearrange("p (m c) -> p m c", m=2)[
                    :, :, p0:p0 + cw]
                nc.scalar.activation(hovw, gvw, AF.Tanh)

            # ---------------- internal levels ----------------
            for lv in range(1, 7):
                Lp, Lc = LS[lv], LS[lv - 1]
                hp = h_sb[lv - 1][:].rearrange("p (m c) -> p m c", m=2)
                hsv = hs_sb[lv][:].rearrange("p (m c) -> p m c", m=2)
                fsv = fs_sb[lv][:].rearrange("p (m c) -> p m c", m=2)
                hcv = h_sb[lv][:].rearrange("p (m c) -> p m c", m=2)

                # h_sum = h_left + h_right (contiguous halves)
                nc.vector.tensor_add(hsv[:, :, :],
                                     hp[:, :, 0:Lp], hp[:, :, Lp:Lc])

                # --- f gates, parent chunks (left+right kids in one psum) ---
                for p0 in range(0, Lp, PCH):
                    pw = min(PCH, Lp - p0)
                    psF = pb.tile([128, 1024], f32, tag="psB")
                    xsl = xpv[lv][:, :, NOFF[lv] - NOFF[lv] + p0:p0 + pw]
                    xsl = xpv[lv][:, :, p0:p0 + pw]
                    x2f_k = x2f_sb[lv][:].rearrange(
                        "p (h c) -> p h c", h=2)[:, :, p0:p0 + pw]
                    for m in range(2):
                        ov = psF[:, m * 512:(m + 1) * 512].rearrange(
                            "p (h c) -> p h c", h=2)[:, :, 0:pw]
                        mc = slice(m * 128, (m + 1) * 128)
                        if USE_BCAST:
                            xb = xsl.unsqueeze(2).to_broadcast(
                                [128, 2, 2, pw])
                            mm(ov, wv[:, :, mc], xb,
                               start=True, stop=False, dr=True)
                            mm(ov, w2f_sb[:, mc], x2f_k,
                               start=False, stop=False)
                            for k in range(2):
                                hr = hp[:, k, 0:Lc].rearrange(
                                    "p (h c) -> p h c", h=2)[:, :, p0:p0 + pw]
                                mm(ov, uv[:, k, mc], hr,
                                   start=False, stop=(k == 1))
                        else:
                            for hh in range(2):
                                o = psF[:, m * 512 + hh * 256:
                                        m * 512 + hh * 256 + pw]
                                mm(o, wv[:, :, mc], xsl,
                                   start=True, stop=False, dr=True)
                                mm(o, w2f_sb[:, mc],
                                   x2f_k[:, hh, :],
                                   start=False, stop=False)
                                for k in range(2):
                                    mm(o, uv[:, k, mc],
                                       hp[:, k, hh * Lp + p0:
                                          hh * Lp + p0 + pw],
                                       start=False, stop=(k == 1))
                    fe = work.tile([128, 1024], bf16, tag="fe")
                    for m in range(2):
                        iv = psF[:, m * 512:(m + 1) * 512].rearrange(
                            "p (h c) -> p h c", h=2)[:, :, 0:pw]
                        ev = fe[:, m * 512:(m + 1) * 512].rearrange(
                            "p (h c) -> p h c", h=2)[:, :, 0:pw]
                        nc.scalar.activation(ev, iv, AF.Sigmoid)
                    for m in range(2):
                        fhL = work.tile([128, 256], bf16, tag="fhL")
                        fhR = work.tile([128, 256], bf16, tag="fhR")
                        nc.vector.tensor_mul(
                            fhL[:, 0:pw], fe[:, m * 512:m * 512 + pw],
                            hp[:, m, p0:p0 + pw])
                        nc.vector.tensor_mul(
                            fhR[:, 0:pw],
                            fe[:, m * 512 + 256:m * 512 + 256 + pw],
                            hp[:, m, Lp + p0:Lp + p0 + pw])
                        nc.vector.tensor_add(
                            fsv[:, m, p0:p0 + pw], fhL[:, 0:pw],
                            fhR[:, 0:pw])

                # --- iu, parent chunks ---
                for p0 in range(0, Lp, PCH):
                    pw = min(PCH, Lp - p0)
                    psIU = pa.tile([128, 1024], f32, tag="psA")
                    for m in range(4):
                        o = psIU[:, m * 256:m * 256 + pw]
                        mc = slice(256 + m * 128, 256 + (m + 1) * 128)
                        mm(o, wv[:, :, mc], xpv[lv][:, :, p0:p0 + pw],
                           start=True, stop=False, dr=True)
                        mm(o, w2iu_sb[:, m * 128:(m + 1) * 128],
                           x2iu_sb[lv][:, p0:p0 + pw],
                           start=False, stop=False)
                        for k in range(2):
                            mm(o, uv[:, k, mc], hsv[:, k, p0:p0 + pw],
                               start=False, stop=(k == 1))
                    si = work.tile([128, 512], bf16, tag="si")
                    tu = work.tile([128, 512], bf16, tag="tu")
                    sivw = si[:].rearrange("p (m c) -> p m c", m=2)[
                        :, :, 0:pw]
                    tuvw = tu[:].rearrange("p (m c) -> p m c", m=2)[
                        :, :, 0:pw]
                    piv = psIU[:, 0:512].rearrange(
                        "p (m c) -> p m c", m=2)[:, :, 0:pw]
                    puv = psIU[:, 512:1024].rearrange(
                        "p (m c) -> p m c", m=2)[:, :, 0:pw]
                    nc.scalar.activation(sivw, piv, AF.Sigmoid)
                    nc.scalar.activation(tuvw, puv, AF.Tanh)
                    g = work.tile([128, 512], bf16, tag="g")
                    g2 = work.tile([128, 512], bf16, tag="g2")
                    nc.vector.tensor_mul(g[:, :], si[:, :], tu[:, :])
                    gv = g[:].rearrange("p (m c) -> p m c", m=2)[:, :, 0:pw]
                    g2v = g2[:].rearrange("p (m c) -> p m c", m=2)[:, :, 0:pw]
                    nc.vector.tensor_add(g2v, gv, fsv[:, :, p0:p0 + pw])
                    nc.scalar.activation(hcv[:, :, p0:p0 + pw], g2v, AF.Tanh)

            # ---------------- roots -> output (host transposes) ----------
            nc.sync.dma_start(out=out_d.ap(), in_=h_sb[6][:, :])

    nc.compile()
    return nc


def prep_inputs(tokens, dep, idx2vec, q, W, U, D, b):
    """Host-side prep: per-core input maps with pre-gathered fp8 x streams."""
    import ml_dtypes

    bf = ml_dtypes.bfloat16
    f8 = ml_dtypes.float8_e4m3fn
    tokens = np.asarray(tokens, np.int32)
    dep = np.asarray(dep, np.int32)
    idx2vec = np.asarray(idx2vec, np.float32)
    q = np.asarray(q, np.float32)
    W = np.asarray(W, np.float32)
    U = np.asarray(U, np.float32)
    D = np.asarray(D, np.float32)
    b = np.asarray(b, np.float32)

    emb8 = idx2vec.astype(f8)

    WT = np.ascontiguousarray(W.T)            # [300, 768]
    UT = np.ascontiguousarray(U.T)            # [256, 768]
    qD = q @ D.T                              # [10, 768]
    qdiu = qD[:, 256:] + b[None, 256:] / 2.0  # [10, 512]
    qdf = qD[:, :256] + b[None, :256]         # [10, 256]
    leafconst = qD[-1, 256:] + b[256:]        # [512]

    wk = np.stack([WT[0:128], WT[128:256]])           # [2, 128, 768]
    wk = np.ascontiguousarray(wk.transpose(1, 0, 2)).astype(f8)

    def res8(v):
        a = v.astype(f8).astype(np.float32)
        return a, (v - a)

    w2iu = np.zeros((66, 512), np.float32)
    w2iu[0:44] = WT[256:300, 256:768]
    w2iu[44:54], w2iu[54:64] = res8(qdiu)
    w2iu[64], w2iu[65] = res8(leafconst[None, :])
    w2iu = w2iu.astype(f8)

    w2f = np.zeros((64, 256), np.float32)
    w2f[0:44] = WT[256:300, 0:256]
    w2f[44:54], w2f[54:64] = res8(qdf)
    w2f = w2f.astype(f8)

    uk = np.stack([UT[0:128], UT[128:256]])
    uk = np.ascontiguousarray(uk.transpose(1, 0, 2)).astype(bf)

    shared = dict(wk=wk.reshape(128, -1), w2iu=w2iu,
                  w2f=w2f, uk=uk.reshape(128, -1))

    P = PERM
    pnode = np.maximum((P % 127 - 1) // 2, 0) + (P // 127) * 127  # parent ids
    tt = P // 127
    n = P % 127
    lkid = tt * 127 + 2 * n + 1
    rkid = tt * 127 + 2 * n + 2
    internal = (n < 63)
    isleaf = ~internal

    per_core = []
    for c in range(NCORES):
        tokf = tokens[c * BT:(c + 1) * BT].reshape(-1)
        depf = dep[c * BT:(c + 1) * BT].reshape(-1)
        G8 = emb8[tokf[P]]                    # [NN, 300] fp8
        GP8 = emb8[tokf[pnode]]               # parent rows (for f gates)

        m = dict(shared)
        for lv in range(7):
            s = slice(NOFF[lv], NOFF[lv + 1])
            gs = G8[s]
            xp = np.stack([gs[:, 0:128].T, gs[:, 128:256].T])  # [2,128,L]
            m[f"xp{lv}"] = np.ascontiguousarray(
                xp.transpose(1, 0, 2)).reshape(128, -1)

            a = np.zeros((66, LS[lv]), np.float32)
            a[0:44] = gs[:, 256:300].T.astype(np.float32)
            if lv > 0:
                dl = depf[lkid[s]]
                dr = depf[rkid[s]]
                oh = (dl[None, :] == np.arange(10)[:, None]).astype(
                    np.float32)
                oh += (dr[None, :] == np.arange(10)[:, None])
                a[44:54] = oh
                a[54:64] = oh
            else:
                a[64] = 1.0
                a[65] = 1.0
            m[f"x2iu{lv}"] = np.ascontiguousarray(a.astype(f8))

            if lv > 0:
                sc = slice(NOFF[lv - 1], NOFF[lv])
                gp = GP8[sc]
                af = np.zeros((64, LS[lv - 1]), np.float32)
                af[0:44] = gp[:, 256:300].T.astype(np.float32)
                dc = depf[P[sc]]
                af[44:54] = (dc[None, :] == np.arange(10)[:, None])
                af[54:64] = af[44:54]
                m[f"x2f{lv}"] = np.ascontiguousarray(af.astype(f8))
        per_core.append(m)
    return per_core


_NC_CACHE = {}
TRACE = False
LAST = None


def _get_nc():
    if "nc" not in _NC_CACHE:
        _NC_CACHE["nc"] = build_nc()
    return _NC_CACHE["nc"]


def kernel(tokens, dep, idx2vec, q, W, U, D, b):
    global LAST
    from concourse.bass_utils import run_bass_kernel_spmd

    nc = _get_nc()
    in_maps = prep_inputs(tokens, dep, idx2vec, q, W, U, D, b)
    res = run_bass_kernel_spmd(nc, in_maps, list(range(NCORES)), trace=TRACE)
    LAST = res
    outs = []
    for i in range(NCORES):
        arr = np.asarray(res.results[i]["out"], np.float32)  # [128, 2*BT]
        h = np.empty((BT, 256), np.float32)
        h[:, 0:128] = arr[:, 0:BT].T
        h[:, 128:256] = arr[:, BT:2 * BT].T
        outs.append(h)
    return np.concatenate(outs, axis=0)
